# revision 1
# baseline (speedup 1.0000x reference)
"""GCNConv Trainium2 kernel: out = (segsum_{dst}(x[src]*norm[src]) @ W) * norm[dst] + bias.

Distribution: dst-nodes sharded across 8 NeuronCores (12500 each). Each core
gathers x rows for its incoming edges straight from HBM via dma_gather
(int16 indices, 4 source chunks of 25000 rows), segment-sums them on the
tensor engine via an on-the-fly selection matrix S[e,seg] =
(iota==dst_local)*norm_src, projects through W, scales and biases.
Host does index bucketing only. All arithmetic fp32.

Per-call gather is capped at 1024 indices by the SWDGE descriptor ring, and
descriptor generation costs ~8ns/idx on the Q7 (the kernel's bottleneck), so
calls are one (dst-block, src-chunk) group each, sized to the max count over
cores (rounded to 16) to minimize padding generation.
"""

import numpy as np

N = 100000
C = 128
NC_ = 8
NPC = N // NC_            # 12500 dst nodes per core
BLK = 128
NBLK = (NPC + BLK - 1) // BLK   # 98 blocks (last has 84 rows)
LAST_ROWS = NPC - (NBLK - 1) * BLK  # 84
# src chunk boundaries: int16 gather indices reach 32767 rows. These splits
# give 6+6+6+1 = 19 gather tiles per dst block on the benchmark edge
# distribution (Q7 descriptor generation cost is proportional to tiles); the
# capacities (TBC) are still derived from the actual data, so other
# distributions just get bigger capacities, not wrong answers.
CHUNK_BOUNDS = [0, 31000, 62500, 95267, 100000]
NCHUNK = len(CHUNK_BOUNDS) - 1
NBUF = 4                  # msgs ring depth (per chunk, in block-groups)
GROUPS = [2, 2, 2, 4]     # blocks merged per gather call, per chunk

_prog_cache = {}


def _build_program(TBC, NUMS):
    """NUMS[b][c]: static per-call index count (max over cores, rounded up to
    16). Slots beyond it in the TBC_c-tile buffer are never written and are
    cancelled by S==0; keeping num_idxs == true generated count keeps the
    SWDGE ring reservation in decode consistent with what the Q7 pushes."""
    import concourse.bacc as bacc
    import concourse.mybir as mybir
    import concourse.tile as tile
    from concourse.library_config import mlp
    from contextlib import ExitStack

    f32 = mybir.dt.float32
    TB = sum(TBC)
    co = [0]
    for t in TBC:
        co.append(co[-1] + t)

    idx_cols = 0
    for c in range(NCHUNK):
        g = GROUPS[c]
        qb = 0
        while qb < NBLK:
            qsz = min(g, NBLK - qb)
            idx_cols += (128 * TBC[c] * (qsz - 1) + NUMS[qb + qsz - 1][c]) // 16
            qb += g

    nc = bacc.Bacc("TRN2", target_bir_lowering=False, debug=False)
    xb_d = nc.dram_tensor("xb", [N, C], f32, kind="ExternalInput")
    idx_d = nc.dram_tensor("idx", [128, idx_cols], mybir.dt.int16, kind="ExternalInput")
    dstl_d = nc.dram_tensor("dstl", [128, NBLK * TB], f32, kind="ExternalInput")
    nsrc_d = nc.dram_tensor("nsrc", [128, NBLK * TB], f32, kind="ExternalInput")
    ndst_d = nc.dram_tensor("ndst", [128, NBLK], f32, kind="ExternalInput")
    w_d = nc.dram_tensor("w", [C, C], f32, kind="ExternalInput")
    biasb_d = nc.dram_tensor("biasb", [128, C], f32, kind="ExternalInput")
    iota_d = nc.dram_tensor("iota", [128, 128], f32, kind="ExternalInput")
    out_d = nc.dram_tensor("out", [NPC, C], f32, kind="ExternalOutput")

    nc.gpsimd.load_library(mlp)
    with tile.TileContext(nc) as tc, ExitStack() as ctx:
        const = ctx.enter_context(tc.tile_pool(name="const", bufs=1))

        idx_sb = const.tile([128, idx_cols], mybir.dt.int16)
        nc.sync.dma_start(idx_sb[:], idx_d.ap()[:])
        dstl_sb = const.tile([128, NBLK * TB], f32)
        nc.sync.dma_start(dstl_sb[:], dstl_d.ap()[:])
        nsrc_sb = const.tile([128, NBLK * TB], f32)
        nc.sync.dma_start(nsrc_sb[:], nsrc_d.ap()[:])
        ndst_sb = const.tile([128, NBLK], f32)
        nc.sync.dma_start(ndst_sb[:], ndst_d.ap()[:])
        w_sb = const.tile([C, C], f32)
        nc.sync.dma_start(w_sb[:], w_d.ap()[:])
        biasb_sb = const.tile([128, C], f32)
        nc.sync.dma_start(biasb_sb[:], biasb_d.ap()[:])
        iota_sb = const.tile([128, 128], f32)
        nc.sync.dma_start(iota_sb[:], iota_d.ap()[:])

        # Persistent msgs ring buffers (per chunk), memset once: -1-padded
        # gather slots are never written by the DMA, so they must start (and
        # then stay) finite; S==0 cancels their contribution exactly.
        mpool = ctx.enter_context(tc.tile_pool(name="msgs", bufs=1))
        bufs = {}
        for c in range(NCHUNK):
            width = TBC[c] * C * GROUPS[c]
            for i in range(NBUF):
                t = mpool.tile([128, width], f32, tag=f"mb{c}_{i}")
                nc.vector.memset(t[:], 0.0)
                bufs[(c, i)] = t

        spool = ctx.enter_context(tc.tile_pool(name="sel", bufs=12))
        apool = ctx.enter_context(tc.tile_pool(name="aggT", bufs=3))
        opool = ctx.enter_context(tc.tile_pool(name="outt", bufs=3))
        accp = ctx.enter_context(tc.tile_pool(name="acc", bufs=3, space="PSUM"))
        projp = ctx.enter_context(tc.tile_pool(name="proj", bufs=2, space="PSUM"))

        idx_col = 0
        for b in range(NBLK):
            for c in range(NCHUNK):
                g = GROUPS[c]
                if b % g == 0:
                    qsz = min(g, NBLK - b)
                    L = 128 * TBC[c] * (qsz - 1) + NUMS[b + qsz - 1][c]
                    nt = (L + 127) // 128
                    m = bufs[(c, (b // g) % NBUF)]
                    nc.gpsimd.dma_gather(
                        out_ap=m[:, : nt * C].rearrange("p (t f) -> p t f", f=C),
                        in_ap=xb_d.ap()[CHUNK_BOUNDS[c]:CHUNK_BOUNDS[c + 1], :],
                        idxs_ap=idx_sb[:, idx_col: idx_col + L // 16],
                        num_idxs=L,
                        num_idxs_reg=L,
                        elem_size=C,
                        single_packet=(L <= 1024),
                    )
                    idx_col += L // 16
            acc = accp.tile([128, 128], f32)
            ti = 0
            for c in range(NCHUNK):
                m = bufs[(c, (b // GROUPS[c]) % NBUF)]
                for u in range(TBC[c]):
                    col = b * TB + ti
                    S = spool.tile([128, 128], f32)
                    nc.vector.tensor_scalar(
                        out=S[:],
                        in0=iota_sb[:],
                        scalar1=dstl_sb[:, col:col + 1],
                        scalar2=nsrc_sb[:, col:col + 1],
                        op0=mybir.AluOpType.is_equal,
                        op1=mybir.AluOpType.mult,
                    )
                    uo = u + (b % GROUPS[c]) * TBC[c]
                    nc.tensor.matmul(
                        out=acc[:],
                        lhsT=m[:, uo * C:(uo + 1) * C],
                        rhs=S[:],
                        start=(ti == 0),
                        stop=(ti == TB - 1),
                    )
                    ti += 1
            aggT = apool.tile([128, 128], f32)
            nc.scalar.copy(aggT[:], acc[:])
            proj = projp.tile([128, 128], f32)
            nc.tensor.matmul(out=proj[:], lhsT=aggT[:], rhs=w_sb[:], start=True, stop=True)
            outt = opool.tile([128, C], f32)
            nc.vector.scalar_tensor_tensor(
                out=outt[:],
                in0=proj[:],
                scalar=ndst_sb[:, b:b + 1],
                in1=biasb_sb[:],
                op0=mybir.AluOpType.mult,
                op1=mybir.AluOpType.add,
            )
            rows = LAST_ROWS if b == NBLK - 1 else 128
            nc.sync.dma_start(out_d.ap()[b * 128: b * 128 + rows, :], outt[:rows, :])
    nc.compile()
    return nc


def _preprocess(x, norm, weight, bias, edge_src, edge_dst):
    src = np.asarray(edge_src).astype(np.int64, copy=False).ravel()
    dst = np.asarray(edge_dst).astype(np.int64, copy=False).ravel()
    E = src.size
    normf = np.asarray(norm, dtype=np.float32).ravel()

    core = dst // NPC
    rem = dst - core * NPC
    blk = rem >> 7
    dstl = (rem & 127).astype(np.float32)
    bounds = np.asarray(CHUNK_BOUNDS, dtype=np.int64)
    chunk = np.searchsorted(bounds, src, side="right") - 1
    lsrc = (src - bounds[chunk]).astype(np.int16)
    nsrcv = normf[src]

    key = ((core * NBLK + blk) * NCHUNK + chunk).astype(np.int64)
    order = np.argsort(key, kind="stable")
    key_s = key[order]
    counts = np.bincount(key, minlength=NC_ * NBLK * NCHUNK)
    cnt3 = counts.reshape(NC_, NBLK, NCHUNK)
    TBC = tuple(max(1, int(np.ceil(cnt3[:, :, c].max() / 128))) for c in range(NCHUNK))
    TB = sum(TBC)

    cap = np.array([TBC[c] * 128 for c in range(NCHUNK)], dtype=np.int64)
    caps = np.tile(cap, NC_ * NBLK)
    g_start = np.concatenate([[0], np.cumsum(caps)[:-1]])
    starts_e = np.concatenate([[0], np.cumsum(counts)[:-1]])
    rank = np.arange(E, dtype=np.int64) - starts_e[key_s]
    slot = g_start[key_s] + rank

    TOT = int(caps.sum())  # NC_*NBLK*TB*128
    # valid-0 padding: pad slots gather row 0 of the chunk (harmless, nsrc=0
    # cancels); -1 tail-trim padding crashes the ucode on this image.
    p_lsrc = np.zeros(TOT, np.int16)
    p_dstl = np.zeros(TOT, np.float32)
    p_nsrc = np.zeros(TOT, np.float32)
    p_lsrc[slot] = lsrc[order]
    p_dstl[slot] = dstl[order]
    p_nsrc[slot] = nsrcv[order]

    P_lsrc = p_lsrc.reshape(NC_, NBLK, TB * 128)
    P_dstl = p_dstl.reshape(NC_, NBLK, TB * 128)
    P_nsrc = p_nsrc.reshape(NC_, NBLK, TB * 128)

    xb = np.ascontiguousarray(np.asarray(x, dtype=np.float32))
    iota = np.broadcast_to(np.arange(128, dtype=np.float32), (128, 128)).copy()
    biasb = np.broadcast_to(np.asarray(bias, np.float32), (128, C)).copy()
    w = np.asarray(weight, dtype=np.float32)

    nd_full = np.zeros((NC_, NBLK * 128), np.float32)
    nd_full[:, :NPC] = normf.reshape(NC_, NPC)
    ndst = nd_full.reshape(NC_, NBLK, 128).transpose(0, 2, 1).copy()  # [NC,128,NBLK]

    # Static per-call index count: max over cores, rounded up to 16 (the idx
    # wrap granularity). Keeps decode's ring reservation == Q7 pushes while
    # skipping most capacity padding.
    NUMS = [[int(-(-max(1, int(cnt3[:, b, c].max())) // 16) * 16) for c in range(NCHUNK)]
            for b in range(NBLK)]

    # wrapped idx layout: within each (b, c) call of L=NUMS[b][c] indices,
    # index j lives at [j%16, j//16], replicated across the 8 Q7 groups.
    co = np.concatenate([[0], np.cumsum(TBC)])
    in_maps = []
    for k in range(NC_):
        segs = []
        for b in range(NBLK):
            for c in range(NCHUNK):
                g = GROUPS[c]
                if b % g == 0:
                    qsz = min(g, NBLK - b)
                    parts = [P_lsrc[k, b + j, co[c] * 128:co[c] * 128 + 128 * TBC[c]]
                             for j in range(qsz - 1)]
                    parts.append(P_lsrc[k, b + qsz - 1,
                                        co[c] * 128:co[c] * 128 + NUMS[b + qsz - 1][c]])
                    a = np.concatenate(parts)
                    segs.append(a.reshape(-1, 16).T)
        idx16 = np.concatenate(segs, axis=1)
        idx_w = np.tile(idx16, (8, 1))

        dstl_k = P_dstl[k].reshape(NBLK, TB, 128).transpose(2, 0, 1).reshape(128, NBLK * TB)
        nsrc_k = P_nsrc[k].reshape(NBLK, TB, 128).transpose(2, 0, 1).reshape(128, NBLK * TB)

        in_maps.append({
            "xb": xb,
            "idx": np.ascontiguousarray(idx_w),
            "dstl": np.ascontiguousarray(dstl_k),
            "nsrc": np.ascontiguousarray(nsrc_k),
            "ndst": np.ascontiguousarray(ndst[k]),
            "w": w,
            "biasb": biasb,
            "iota": iota,
        })
    return TBC, NUMS, in_maps


def _run(inputs, trace=False, trace_kwargs=None):
    from concourse.bass_utils import run_bass_kernel_spmd

    TBC, NUMS, in_maps = _preprocess(**inputs)
    key = (TBC, tuple(tuple(r) for r in NUMS))
    if key not in _prog_cache:
        _prog_cache[key] = _build_program(TBC, NUMS)
    nc = _prog_cache[key]
    kw = {}
    if trace:
        kw["trace"] = True
        if trace_kwargs:
            kw["trace_kwargs"] = trace_kwargs
    res = run_bass_kernel_spmd(nc, in_maps, core_ids=list(range(NC_)), **kw)
    out = np.concatenate([res.results[k]["out"] for k in range(NC_)], axis=0)
    return out, res


def kernel(**inputs):
    out, _ = _run(inputs, trace=False)
    return out

